# revision 1
# baseline (speedup 1.0000x reference)
"""Trainium2 Bass kernel: LSTM (B=2048, T=1024, I=4, H=16) + sigmoid dense head.

Sharding: pure data parallel, batch split over 8 cores (256 each = 2 chains x 128).
Per chain-step on device:
  - 1 bf16 matmul producing all 4 gate pre-activations spread at partition
    slots {0:f, 32:g, 64:o, 96:i} (zero-padded M=112 lhsT)
  - sigmoid-only activations (tanh via sigma(2x) identities, scalings folded
    into weights on host): ACT1a (f,g,o -> PSUM), ACT1b (i -> SBUF), ACT2
  - 3 DVE fused scalar_tensor_tensor ops (each keeps one PSUM operand to
    avoid the SBUF-source errata) + 1 GPSIMD add for the c-state combine
State scalings: c~ = c/2, h~ = h/2 (absorbed into weights).
Output head: block-diagonal 2*W_d matmul over 8-step groups -> sigma ->
out.T [T, B] in DRAM; final transpose on host.
"""
import sys
sys.path.insert(0, "/opt/trn_rl_repo")
import numpy as np
from contextlib import ExitStack

import concourse.bass as bass
import concourse.tile as tile
from concourse import bacc, mybir

F32 = mybir.dt.float32
BF16 = mybir.dt.bfloat16
AF = mybir.ActivationFunctionType
OP = mybir.AluOpType

B, T, I, H = 2048, 1024, 4, 16
NCORES = 8
BCORE = B // NCORES          # 256
NB = 128                     # batch per chain
NCH = 2                      # chains per core
KD = 21                      # rhs rows: 16 h~ + 4 x
KSLOT = 257                  # Z ring slots (8*32+1: head groups never wrap)
STAGE = 128                  # x staging granularity (steps)

_CACHE = {}


def _emit_core(nc, t_steps):
    wg = nc.dram_tensor("wg", [KD, 112], BF16, kind="ExternalInput").ap()
    bg = nc.dram_tensor("bg", [112, 1], F32, kind="ExternalInput").ap()
    whd = nc.dram_tensor("whd", [128, 8], BF16, kind="ExternalInput").ap()
    bh = nc.dram_tensor("bh", [104, 1], F32, kind="ExternalInput").ap()
    xt = nc.dram_tensor("xt", [t_steps, I, BCORE], BF16, kind="ExternalInput").ap()
    ones = nc.dram_tensor("ones", [1, KSLOT * NB], BF16, kind="ExternalInput").ap()
    h_in = nc.dram_tensor("h_in", [16, BCORE], BF16, kind="ExternalInput").ap()
    c_in = nc.dram_tensor("c_in", [16, BCORE], F32, kind="ExternalInput").ap()
    h_out = nc.dram_tensor("h_out", [16, BCORE], BF16, kind="ExternalOutput").ap()
    c_out = nc.dram_tensor("c_out", [16, BCORE], F32, kind="ExternalOutput").ap()
    ot = nc.dram_tensor("ot", [t_steps, BCORE], F32, kind="ExternalOutput").ap()

    with tile.TileContext(nc) as tc, ExitStack() as ctx:
        const = ctx.enter_context(tc.tile_pool(name="const", bufs=1))
        zpool = ctx.enter_context(tc.tile_pool(name="zp", bufs=1))
        work = ctx.enter_context(tc.tile_pool(name="wk", bufs=4))
        hhp = ctx.enter_context(tc.tile_pool(name="hhp", bufs=3))
        osp = ctx.enter_context(tc.tile_pool(name="osp", bufs=2))
        ghp = ctx.enter_context(tc.tile_pool(name="ghp", bufs=3, space="PSUM"))
        odp = ctx.enter_context(tc.tile_pool(name="odp", bufs=1, space="PSUM"))

        twg = const.tile([KD, 112], BF16)
        tbg = const.tile([112, 1], F32)
        twhd = const.tile([128, 8], BF16)
        tbh = const.tile([104, 1], F32)
        nc.sync.dma_start(twg[:], wg[:])
        nc.sync.dma_start(tbg[:], bg[:])
        nc.sync.dma_start(twhd[:], whd[:])
        nc.sync.dma_start(tbh[:], bh[:])

        # Z rings: rows 0:16 h~ (bf16), rows 16:20 x (bf16)
        z = [zpool.tile([KD, KSLOT * NB], BF16, name=f"z{c}") for c in range(NCH)]
        for c in range(NCH):
            nc.sync.dma_start(z[c][0:16, 0:NB], h_in[:, c * NB:(c + 1) * NB])
            nc.sync.dma_start(z[c][20:21, :], ones[:])

        c_cur = []
        for c in range(NCH):
            ci = work.tile([48, NB], F32, tag=f"c{c}", name=f"ci{c}")
            nc.sync.dma_start(ci[32:48, :], c_in[:, c * NB:(c + 1) * NB])
            c_cur.append(ci)

        def stage_x(c, t0, nsteps):
            s0 = t0 % KSLOT
            runs = []
            if s0 + nsteps <= KSLOT:
                runs.append((s0, t0, nsteps))
            else:
                n1 = KSLOT - s0
                runs.append((s0, t0, n1))
                runs.append((0, t0 + n1, nsteps - n1))
            for (sl, tt, ln) in runs:
                src = xt[tt:tt + ln, :, c * NB:(c + 1) * NB].rearrange("t i b -> i t b")
                dst = z[c][16:20, sl * NB:(sl + ln) * NB].rearrange(
                    "i (s b) -> i s b", s=ln)
                nc.sync.dma_start(dst, src)

        for c in range(NCH):
            stage_x(c, 0, min(STAGE, t_steps))

        od_cur = [None] * NCH

        for t in range(t_steps):
            if t % STAGE == 0 and t + STAGE < t_steps:
                for c in range(NCH):
                    stage_x(c, t + STAGE, min(STAGE, t_steps - t - STAGE))
            sl = t % KSLOT
            nsl = (t + 1) % KSLOT
            z_t = [z[c][:, sl * NB:(sl + 1) * NB] for c in range(NCH)]
            gh = [ghp.tile([112, 2 * NB], F32, tag=f"gh{c}", name=f"gh{c}_{t}")
                  for c in range(NCH)]
            g_ps = [gh[c][:, 0:NB] for c in range(NCH)]
            h_ps = [gh[c][:, NB:2 * NB] for c in range(NCH)]
            v = [work.tile([48, NB], F32, tag=f"v{c}", name=f"v{c}_{t}")
                 for c in range(NCH)]
            pb = [work.tile([48, NB], F32, tag=f"pb{c}", name=f"pb{c}_{t}")
                  for c in range(NCH)]
            q = [work.tile([48, NB], F32, tag=f"q{c}", name=f"q{c}_{t}")
                 for c in range(NCH)]
            cn = [work.tile([48, NB], F32, tag=f"c{c}", name=f"c{c}_{t}")
                  for c in range(NCH)]
            u = [work.tile([80, NB], F32, tag=f"u{c}", name=f"u{c}_{t}")
                 for c in range(NCH)]
            for c in range(NCH):
                nc.tensor.matmul(g_ps[c], twg[:], z_t[c][0:KD, :], start=True, stop=True)
            for c in range(NCH):
                nc.scalar.activation(h_ps[c][0:80, :], g_ps[c][0:80, :], AF.Sigmoid)
            for c in range(NCH):
                nc.scalar.activation(v[c][32:48, :], g_ps[c][96:112, :], AF.Sigmoid)
            for c in range(NCH):
                nc.vector.scalar_tensor_tensor(
                    pb[c][32:48, :], h_ps[c][0:16, :], 0.0, c_cur[c][32:48, :],
                    op0=OP.add, op1=OP.mult)
            for c in range(NCH):
                nc.vector.scalar_tensor_tensor(
                    q[c][32:48, :], h_ps[c][32:48, :], 0.5, v[c][32:48, :],
                    op0=OP.subtract, op1=OP.mult)
            for c in range(NCH):
                nc.gpsimd.tensor_tensor(
                    cn[c][32:48, :], q[c][32:48, :], pb[c][32:48, :], op=OP.add)
            for c in range(NCH):
                nc.scalar.activation(u[c][64:80, :], cn[c][32:48, :], AF.Sigmoid,
                                     scale=4.0)
            for c in range(NCH):
                nc.vector.scalar_tensor_tensor(
                    z[c][0:16, nsl * NB:(nsl + 1) * NB],
                    u[c][64:80, :], 0.5, h_ps[c][64:80, :],
                    op0=OP.subtract, op1=OP.mult)
                c_cur[c] = cn[c]

            # output head
            if t % 8 == 7:
                g8 = t // 8
                s0 = (g8 * 8 + 1) % KSLOT
                for c in range(NCH):
                    hh = hhp.tile([128, NB], BF16, tag=f"hh{c}", name=f"hh{c}_{g8}")
                    for j in range(8):
                        nc.sync.dma_start(
                            hh[16 * j:16 * j + 16, :],
                            z[c][0:16, (s0 + j) * NB:(s0 + j + 1) * NB])
                    if g8 % 4 == 0:
                        od_cur[c] = odp.tile([104, NB], F32, tag=f"od{c}",
                                             name=f"od{c}_{g8 // 4}")
                    base = 32 * (g8 % 4)
                    nc.tensor.matmul(od_cur[c][base:base + 8, :], twhd[:], hh[:],
                                     start=True, stop=True, tile_position=(0, base))
                if g8 % 4 == 3:
                    blk = g8 // 4
                    for c in range(NCH):
                        os_ = osp.tile([104, NB], F32, tag=f"os{c}",
                                       name=f"os{c}_{blk}")
                        nc.scalar.activation(os_[:], od_cur[c][:], AF.Sigmoid,
                                             bias=tbh[:])
                        for j in range(4):
                            nc.sync.dma_start(
                                ot[blk * 32 + 8 * j:blk * 32 + 8 * j + 8,
                                   c * NB:(c + 1) * NB],
                                os_[32 * j:32 * j + 8, :])

        fsl = t_steps % KSLOT
        for c in range(NCH):
            nc.sync.dma_start(h_out[:, c * NB:(c + 1) * NB],
                              z[c][0:16, fsl * NB:(fsl + 1) * NB])
            nc.sync.dma_start(c_out[:, c * NB:(c + 1) * NB], c_cur[c][32:48, :])


def _prep_host(W_ih, W_hh, b_ih, b_hh, W_d, b_d):
    # PyTorch gate order blocks of 16: [i, f, g, o]
    Wi, Wf, Wgt, Wo = W_ih[0:16], W_ih[16:32], W_ih[32:48], W_ih[48:64]
    Ui, Uf, Ugt, Uo = W_hh[0:16], W_hh[16:32], W_hh[32:48], W_hh[48:64]
    bb = b_ih + b_hh
    bi, bf, bgt, bo = bb[0:16], bb[16:32], bb[32:48], bb[48:64]

    wg = np.zeros((KD, 112), np.float32)
    bg = np.zeros((112, 1), np.float32)

    def put(base, Wx, Ux, bx, scale):
        wg[0:16, base:base + 16] = (2.0 * scale) * Ux.T   # h~ = h/2
        wg[16:20, base:base + 16] = scale * Wx.T
        wg[20, base:base + 16] = scale * bx
        bg[base:base + 16, 0] = scale * bx

    put(0, Wf, Uf, bf, 1.0)
    put(32, Wgt, Ugt, bgt, 2.0)   # sigma(2 glin)
    put(64, Wo, Uo, bo, 1.0)
    put(96, Wi, Ui, bi, 1.0)

    whd = np.zeros((128, 8), np.float32)
    for j in range(8):
        whd[16 * j:16 * j + 16, j] = 2.0 * W_d[0]
    bh = np.full((104, 1), float(b_d[0]), np.float32)
    return wg, bg, whd, bh


def _get_compiled(t_steps):
    key = ("nc", t_steps)
    if key not in _CACHE:
        nc = bacc.Bacc("TRN2", target_bir_lowering=False, debug=False)
        _emit_core(nc, t_steps)
        nc.compile()
        _CACHE[key] = nc
    return _CACHE[key]


def kernel(x, W_ih, W_hh, b_ih, b_hh, W_d, b_d, _trace=False, _t_steps=T):
    import ml_dtypes
    from concourse.bass_utils import run_bass_kernel_spmd

    x = np.asarray(x, dtype=np.float32)
    ts = _t_steps
    wg, bg, whd, bh = _prep_host(
        np.asarray(W_ih, np.float32), np.asarray(W_hh, np.float32),
        np.asarray(b_ih, np.float32), np.asarray(b_hh, np.float32),
        np.asarray(W_d, np.float32), np.asarray(b_d, np.float32))
    wg16 = wg.astype(ml_dtypes.bfloat16)
    whd16 = whd.astype(ml_dtypes.bfloat16)

    # x [B, ts, I] -> [ts, I, B] bf16
    xtr16 = np.ascontiguousarray(
        x[:, 0:ts, :].transpose(1, 2, 0)).astype(ml_dtypes.bfloat16)

    CH = 512 if ts % 512 == 0 else ts
    nchunk = ts // CH
    nc = _get_compiled(CH)
    _ONES = np.ones((1, KSLOT * NB), ml_dtypes.bfloat16)
    h_st = [np.zeros((16, BCORE), ml_dtypes.bfloat16) for _ in range(NCORES)]
    c_st = [np.zeros((16, BCORE), np.float32) for _ in range(NCORES)]
    out = np.empty((B, ts, 1), np.float32)
    total_ns = 0
    for ck in range(nchunk):
        in_maps = []
        for cix in range(NCORES):
            in_maps.append({
                "wg": wg16, "bg": bg, "whd": whd16, "bh": bh,
                "ones": _ONES, "h_in": h_st[cix], "c_in": c_st[cix],
                "xt": np.ascontiguousarray(
                    xtr16[ck * CH:(ck + 1) * CH, :,
                          cix * BCORE:(cix + 1) * BCORE]),
            })
        res = run_bass_kernel_spmd(nc, in_maps, core_ids=list(range(NCORES)),
                                   trace=_trace)
        for cix in range(NCORES):
            out[cix * BCORE:(cix + 1) * BCORE,
                ck * CH:(ck + 1) * CH, 0] = res.results[cix]["ot"].T
            h_st[cix] = res.results[cix]["h_out"]
            c_st[cix] = res.results[cix]["c_out"]
        if res.exec_time_ns:
            total_ns += res.exec_time_ns
    kernel._last_exec_ns = total_ns or None
    return out

